# revision 39
# baseline (speedup 1.0000x reference)
"""Trainium2 Bass kernel for nn_CrossfusionBidirectional.

Sharding: 8 cores = (batch b in {0,1}) x (query-row quarter qi in {0..3}).
Each core computes output rows [qi*784, (qi+1)*784) of batch b with zero
cross-core communication; the host concatenates the 8 slices.

v2: fp8e4 DoubleRow tensor-engine pipeline for every projection
(proj/pl1/K/V), with k-subtile pairs packed in the free dim of shared
SBUF tiles so one DR matmul covers 256 contraction rows at 0.5
cycles/col.  Probability-side attention is fp8 too: exp -> bf16, the
rel-pos multiply writes fp8 pair tiles, and both the AV matmuls and the
softmax denominators run DoubleRow against fp8 V / all-ones tiles
(killing the Pool pair-add tree).  Activation-table eras are scheduled
globally (Sqrt stats prepass -> Gelu chunk loop -> Exp attention ->
Gelu/Sigmoid tail) so the ACT engine loads each table once.  Rel-pos
bias is stored as exp(B) in fp8 and DMA'd per head as one [128,25,784]
tile.  Per-matrix fp8 weight scales are folded into the PSUM-evacuation
copies (activation scale / tensor_scalar_mul), K biases drop by softmax
shift invariance, V biases fold into output-projection biases, LN
affines fold into downstream weights host-side.
"""

import numpy as np

B, L, C, HEADS = 2, 3136, 384, 3
H, H2 = 56, 28
L2 = L // 4
HD = C // HEADS
EPS = 1e-5
NCORES = 8
QPC = L // 4          # 784 query rows per core
CT = C // 128         # 3 feature tiles
NCH, CHW = 7, 448     # (legacy) chunk count hint
CHUNKS = [(i * 512, 512) for i in range(6)] + [(3072, 64)]  # 128-aligned
NQC, QC = 2, 392      # per-core query chunking
TOK2, TT2 = 7, 112    # low-res token tiling (784 = 7*112)
JTS = [(i * 128, 128) for i in range(24)] + [(3072, 64)]   # key tiles
NPAIR = 12            # 24 full key tiles -> 12 DoubleRow pairs

_COMPILED = None
_COMPILED_KEY = None
PHASE_MARKS = []

F8MAX = 240.0
QSC = 64.0            # fp8 q / logB pre-scale; exp() applies 1/QSC


def _q8(a, scale=1.0):
    import ml_dtypes
    return np.clip(np.asarray(a, np.float32) * scale, -F8MAX, F8MAX).astype(
        ml_dtypes.float8_e4m3)


def _pack3(wT, scale):
    """wT [384, out] f32 -> [128, 3, out] fp8 (k-subtiles in middle dim)."""
    out = wT.shape[1]
    w = np.ascontiguousarray(
        np.asarray(wT, np.float32).reshape(3, 128, out).transpose(1, 0, 2))
    return _q8(w, scale)


def _host_prep(inp):
    f32 = np.float32
    g = {}
    scale = f32(HD ** -0.5)
    n1w, n1b = inp["n1_w"].astype(f32), inp["n1_b"].astype(f32)
    n2w, n2b = inp["n2_w"].astype(f32), inp["n2_b"].astype(f32)

    def fold_in(w, b, lnw, lnb):
        return (w * lnw[None, :]).astype(f32), (b + w @ lnb).astype(f32)

    wqh, bqh = fold_in(inp["wqh_w"], inp["wqh_b"], n2w, n2b)
    wkh, _ = fold_in(inp["wkh_w"], inp["wkh_b"], n1w, n1b)
    wvh, bvh = fold_in(inp["wvh_w"], inp["wvh_b"], n1w, n1b)
    wql, bql = fold_in(inp["wql_w"], inp["wql_b"], n1w, n1b)
    pl2w = inp["pl2_w"].astype(f32)
    pl2b_ = inp["pl2_b"].astype(f32)
    # fold pl2 into the low-branch K/V projections; the K-side pl2 bias
    # drops by softmax shift invariance, the V-side folds into folb below
    wkl = (inp["wkl_w"] @ pl2w).astype(f32)
    wvl = (inp["wvl_w"] @ pl2w).astype(f32)
    bvl = (inp["wvl_b"] + inp["wvl_w"] @ pl2b_).astype(f32)

    import ml_dtypes
    bf16 = ml_dtypes.bfloat16

    # q projections stay bf16 (own-quarter bf16 LN outputs feed them)
    g["wqhT"], g["bqh"] = (wqh.T * scale).astype(bf16), bqh * scale
    g["wqlT"], g["bql"] = (wql.T * scale).astype(bf16), bql * scale

    # fp8 packed K/V weights [128, 3, 384] + scales
    scales = {}
    for nm, w in (("kh", wkh.T), ("kl", wkl.T), ("vh", wvh.T), ("vl", wvl.T)):
        s = float(192.0 / np.abs(w).max())
        scales[nm] = s
        g[nm + "P"] = _pack3(w, s)

    pl1L, pl1R = inp["pl1_w"][:, :C], inp["pl1_w"][:, C:]
    pl1Lw, _ = fold_in(pl1L, np.zeros(C, f32), n2w, n2b)
    pl1Rw, _ = fold_in(pl1R, np.zeros(C, f32), n1w, n1b)
    wLT, wRT = pl1Lw.T, pl1Rw.T            # [384(in), 384(out)]
    s_pl1 = float(192.0 / max(np.abs(wLT).max(), np.abs(wRT).max()))
    scales["pl1"] = s_pl1
    # pairs over in-blocks (0,1) of each half + the mixed (p2up2, p1n2) pair
    g["pl1LP"] = _q8(np.stack([wLT[0:128], wLT[128:256]], 1), s_pl1)
    g["pl1RP"] = _q8(np.stack([wRT[0:128], wRT[128:256]], 1), s_pl1)
    g["pl1MX"] = _q8(np.stack([wRT[256:384], wLT[256:384]], 1), s_pl1)
    g["pl1b"] = (inp["pl1_b"] + pl1L @ n2b + pl1R @ n1b).astype(f32)

    gh, gl = f32(inp["gamma_h"][0]), f32(inp["gamma_l"][0])
    g["fohT"] = (inp["foh_w"].T * gh).astype(bf16)
    g["fohb"] = ((inp["foh_b"] + inp["foh_w"] @ bvh) * gh).astype(f32)
    g["folT"] = (inp["fol_w"].T * gl).astype(bf16)
    g["folb"] = ((inp["fol_b"] + inp["fol_w"] @ bvl) * gl).astype(f32)

    g["g1LT"] = inp["g1_w"][:, :C].T.copy().astype(f32)
    g["g1RT"] = inp["g1_w"][:, C:].T.copy().astype(f32)
    g["g1b"] = inp["g1_b"].astype(f32)
    g["g2T"] = inp["g2_w"].T.copy().astype(f32)   # [384, 1]
    g["g2b"] = inp["g2_b"].astype(f32)            # [1]

    ffL, ffR = inp["ff_w"][:, :C], inp["ff_w"][:, C:]
    g["ffLT"] = ffL.T.copy().astype(f32)
    g["ffPT"] = (ffL + ffR).T.copy().astype(f32)
    g["ffb"] = inp["ff_b"].astype(f32)

    # proj: [768, 384] -> [128, 6, 384] fp8 (p2 itself is fp8, unscaled)
    projT = inp["proj_w"].T.astype(f32)
    s_proj = float(192.0 / np.abs(projT).max())
    scales["proj"] = s_proj
    g["projP"] = _q8(np.ascontiguousarray(
        projT.reshape(6, 128, C).transpose(1, 0, 2)), s_proj)
    # LN after proj is scale-invariant, so the bias rides at s_proj scale
    g["projb"] = (inp["proj_b"].astype(f32) * s_proj).astype(bf16)
    g["scales"] = scales
    # zero-mean shortcut for the n1 LayerNorms is exact iff the pen affine
    # is trivial (upsample rows are convex; LN'd xh has zero channel-mean)
    assert np.all(inp["pen_w"] == 1.0) and np.all(inp["pen_b"] == 0.0), (
        "kernel compiled with the zero-mean n1-LN shortcut; pen affine "
        "must be trivial"
    )

    # own-quarter upsample matrix (per core)
    sample_f = (np.arange(H) + 0.5) / 2.0 - 0.5
    wr = 1.0 - np.abs(sample_f[:, None] - np.arange(H2)[None, :])
    wr = np.clip(wr, 0.0, 1.0)
    wr = wr / wr.sum(axis=1, keepdims=True)
    g["Wup"] = np.kron(wr, wr).astype(f32)        # [3136, 784]

    expt = np.exp(inp["rpb_table"].astype(f32))   # [12321, 3]
    rel = np.asarray(inp["rel_index"])            # [L, L] int32
    eB = expt[rel.T].transpose(2, 0, 1)           # [heads, L(keys), L(q)]
    # pad keys 3136 -> 3200 and fold to [heads, 128, 25, L(q)] fp8
    eBp = np.zeros((HEADS, 3200, L), f32)
    eBp[:, :L, :] = eB
    g["expB8"] = _q8(np.ascontiguousarray(
        eBp.reshape(HEADS, 25, 128, L).transpose(0, 2, 1, 3)))  # [h,128,25,L]
    return g


def _build(scales):
    import concourse.bass as bass  # noqa: F401
    import concourse.tile as tile
    from concourse import bacc, mybir

    f32, bf16, f32r = mybir.dt.float32, mybir.dt.bfloat16, mybir.dt.float32r
    fp8 = mybir.dt.float8e4
    AF = mybir.ActivationFunctionType
    OP = mybir.AluOpType
    DR = mybir.MatmulPerfMode.DoubleRow

    r_kh = 1.0 / scales["kh"]
    r_kl = 1.0 / scales["kl"]
    r_vh = 1.0 / scales["vh"]
    r_vl = 1.0 / scales["vl"]
    r_pl1 = 1.0 / scales["pl1"]

    nc = bacc.Bacc("TRN2", target_bir_lowering=False, debug=False,
                   num_devices=NCORES)

    def din(name, shape, dtype=f32):
        return nc.dram_tensor(name, shape, dtype, kind="ExternalInput").ap()

    p1T = din("p1T", [C, L], f32r)
    p1T_own = din("p1T_own", [C, QPC], f32r)
    p2P = din("p2P", [128, 6, L2], fp8)
    WupT_own = din("WupT_own", [L2, QPC], bf16)
    expB8 = din("expB8", [HEADS, NQC, 128, 25, QC], fp8)
    eye128 = din("eye128", [128, 128], bf16)
    ones_row_d = din("ones_row", [1, 128], f32r)
    w_projP = din("w_projP", [128, 6, C], fp8)
    v_projb = din("v_projb", [C], bf16)
    w_qhT, v_bqh = din("w_qhT", [C, C], bf16), din("v_bqh", [C], f32r)
    w_qlT, v_bql = din("w_qlT", [C, C], bf16), din("v_bql", [C], f32r)
    w_khP, w_klP = din("w_khP", [128, 3, C], fp8), din("w_klP", [128, 3, C], fp8)
    w_vhP, w_vlP = din("w_vhP", [128, 3, C], fp8), din("w_vlP", [128, 3, C], fp8)
    w_pl1LP = din("w_pl1LP", [128, 2, C], fp8)
    w_pl1RP = din("w_pl1RP", [128, 2, C], fp8)
    w_pl1MX = din("w_pl1MX", [128, 2, C], fp8)
    v_pl1b = din("v_pl1b", [C])
    w_fohT, v_fohb = din("w_fohT", [C, C], bf16), din("v_fohb", [C])
    w_folT, v_folb = din("w_folT", [C, C], bf16), din("v_folb", [C])
    w_g1LT, w_g1RT = din("w_g1LT", [C, C], f32r), din("w_g1RT", [C, C], f32r)
    v_g1b = din("v_g1b", [C])
    w_g2T, v_g2b = din("w_g2T", [C, 1], f32r), din("v_g2b", [1])
    w_ffLT, w_ffPT = din("w_ffLT", [C, C], f32r), din("w_ffPT", [C, C], f32r)
    v_ffb = din("v_ffb", [C])

    outT = nc.dram_tensor("outT", [C, QPC], f32, kind="ExternalOutput").ap()

    def r32(ap):
        return ap.bitcast(f32r)

    with tile.TileContext(nc) as tc:
      with tc.tile_pool(name="const", bufs=1) as const:
        def load_w3(pool, dram, tag, rows=C):
            ts = []
            for k in range(rows // 128):
                t = pool.tile([128, dram.shape[1]], dram.dtype,
                              tag=f"{tag}_{k}", name=f"{tag}_{k}")
                nc.sync.dma_start(out=t, in_=dram[k * 128:(k + 1) * 128, :])
                ts.append(t)
            return ts

        def load_w3g(pool, dram, tag, rows=C):
            ts = []
            for k in range(rows // 128):
                t = pool.tile([128, dram.shape[1]], dram.dtype,
                              tag=f"{tag}_{k}", name=f"{tag}_{k}")
                nc.gpsimd.dma_start(out=t, in_=dram[k * 128:(k + 1) * 128, :])
                ts.append(t)
            return ts

        def load_b3(pool, dram, tag, dtype=f32):
            ts = []
            v = dram.rearrange("(a b) -> a b", b=1).bitcast(dtype)
            for k in range(CT):
                t = pool.tile([128, 1], dtype, tag=f"{tag}_{k}", name=f"{tag}_{k}")
                nc.sync.dma_start(out=t, in_=v[k * 128:(k + 1) * 128, :])
                ts.append(t)
            return ts

        def load_pk(pool, dram, tag):
            t = pool.tile(list(dram.shape), fp8, tag=tag, name=tag)
            nc.sync.dma_start(out=t, in_=dram)
            return t

        ones_b = const.tile([128, 128], bf16, tag="ones_b", name="ones_b")
        nc.vector.memset(ones_b, 1.0)
        ones8p = const.tile([128, 2, 128], fp8, tag="ones8p", name="ones8p")
        nc.gpsimd.memset(ones8p, 1.0)
        eps_t = const.tile([128, 1], f32, tag="eps_t", name="eps_t")
        nc.vector.memset(eps_t, EPS)
        eye_t = const.tile([128, 128], bf16, tag="eye_t", name="eye_t")
        nc.sync.dma_start(out=eye_t, in_=eye128)

        with tc.tile_pool(name="kvq", bufs=1) as kvq:
            kh = [kvq.tile([128, L], bf16, tag=f"kh{c}", name=f"kh{c}")
                  for c in range(CT)]
            kl = [kvq.tile([128, L], bf16, tag=f"kl{c}", name=f"kl{c}")
                  for c in range(CT)]
            # V pair tiles: 12 x [128, 2, C] + one [64, C] per branch
            vhp = [kvq.tile([128, 2, C], fp8, tag=f"vhp{i}", name=f"vhp{i}")
                   for i in range(NPAIR)]
            vlp = [kvq.tile([128, 2, C], fp8, tag=f"vlp{i}", name=f"vlp{i}")
                   for i in range(NPAIR)]
            vh24 = kvq.tile([64, C], fp8, tag="vh24", name="vh24")
            vl24 = kvq.tile([64, C], fp8, tag="vl24", name="vl24")
            qh = [kvq.tile([128, QPC], bf16, tag=f"qh{c}", name=f"qh{c}")
                  for c in range(CT)]
            ql = [kvq.tile([128, QPC], bf16, tag=f"ql{c}", name=f"ql{c}")
                  for c in range(CT)]

            def vslot(a, vi):
                """(branch, key-128-tile index) -> (tile, slot AP base)"""
                if vi >= 24:
                    return (vh24 if a == 0 else vl24, None)
                t = (vhp if a == 0 else vlp)[vi // 2]
                return t, vi % 2

            with tc.tile_pool(name="mid", bufs=1) as mid:
                u3 = [mid.tile([128, L], bf16, tag=f"u{c}", name=f"u{c}")
                      for c in range(CT)]
                r_u = mid.tile([128, L], bf16, tag="r_u", name="r_u")
                r_p1 = mid.tile([128, L], bf16, tag="r_p1", name="r_p1")
                m_p1 = mid.tile([128, L], bf16, tag="m_p1", name="m_p1")

                with tc.tile_pool(name="early", bufs=1) as early:
                    # token-major LN'd proj output (feeds WupT_own matmul)
                    xn7 = [early.tile([TT2, C], bf16, tag=f"xn{t}", name=f"xn{t}")
                           for t in range(TOK2)]
                    # feature-major copy (feeds separable upsample)
                    xh = [early.tile([128, L2], bf16, tag=f"xhf{c}", name=f"xhf{c}")
                          for c in range(CT)]

                    PHASE_MARKS.append(('ph1', nc._state.next_id()))
                    # ---- Phase 1: x = LN_pen(proj(p2)), token-major;
                    #      proj runs fp8 DoubleRow (LN is scale-invariant) ----
                    with tc.tile_pool(name="ph1s", bufs=1) as ph1s, \
                         tc.tile_pool(name="ph1", bufs=3) as ph1, \
                         tc.tile_pool(name="ph1p", bufs=4, space="PSUM") as ph1p, \
                         tc.tile_pool(name="ph1t", bufs=4, space="PSUM") as ph1t:
                        tproj = load_pk(ph1s, w_projP, "projP")
                        p2s = load_pk(ph1s, p2P, "p2s")
                        projb_row = ph1s.tile([1, C], bf16, tag="projb_row",
                                              name="projb_row")
                        nc.sync.dma_start(
                            out=projb_row,
                            in_=v_projb.rearrange("(a b) -> a b", a=1))
                        one_row = ph1s.tile([1, 128], bf16, tag="one_row",
                                            name="one_row")
                        nc.vector.memset(one_row, 1.0)
                        for tt in range(TOK2):
                            ps = ph1p.tile([TT2, C], f32, tag="ps_x", name="ps_x")
                            sl = slice(tt * TT2, (tt + 1) * TT2)
                            for k in range(3):
                                nc.tensor.matmul(
                                    ps, p2s[:, 2 * k:2 * k + 2, sl],
                                    tproj[:, 2 * k:2 * k + 2, :],
                                    start=(k == 0), stop=False, perf_mode=DR)
                            nc.tensor.matmul(ps, one_row[:, 0:TT2], projb_row,
                                             start=False, stop=True)
                            st = ph1.tile([TT2, 6], f32, tag="bnst", name="bnst")
                            nc.vector.bn_stats(out=st, in_=ps)
                            mv = ph1.tile([TT2, 2], f32, tag="bnmv", name="bnmv")
                            nc.vector.bn_aggr(out=mv, in_=st)
                            sd = ph1.tile([TT2, 1], f32, tag="sd", name="sd")
                            nc.scalar.activation(out=sd, in_=mv[:, 1:2],
                                                 func=AF.Sqrt,
                                                 bias=eps_t[0:TT2], scale=1.0)
                            rr = ph1.tile([TT2, 1], f32, tag="rr", name="rr")
                            nc.vector.reciprocal_approx_fast(out=rr, in_=sd)
                            nmr = ph1.tile([TT2, 1], f32, tag="nmr", name="nmr")
                            nc.vector.scalar_tensor_tensor(
                                out=nmr, in0=mv[:, 0:1], scalar=-1.0, in1=rr,
                                op0=OP.mult, op1=OP.mult)
                            nc.scalar.activation(out=xn7[tt], in_=ps,
                                                 func=AF.Identity,
                                                 bias=nmr, scale=rr)
                            # transpose to feature-major xh
                            for cb in range(CT):
                                pt = ph1t.tile([128, TT2], bf16, tag="ps_t", name="ps_t")
                                nc.tensor.transpose(
                                    pt, xn7[tt][:, cb * 128:(cb + 1) * 128],
                                    eye_t[:TT2, :TT2])
                                nc.vector.tensor_copy(xh[cb][:, sl], pt)

                    PHASE_MARKS.append(('ph2', nc._state.next_id()))
                    # ---- Phase 2: separable x2 bilinear upsample (DVE) ----
                    with tc.tile_pool(name="ph2", bufs=1) as ph2:
                        third = ph2.tile([128, H2 * H], bf16, tag="third", name="third")
                        nc.gpsimd.memset(third, 1.0 / 3.0)
                        tv = third.rearrange("p (i j) -> p i j", j=H)
                        for cb in range(CT):
                            pool_side = (cb == 2)
                            xv = xh[cb].rearrange("p (i j) -> p i j", j=H2)
                            a1 = ph2.tile([128, H2 * H], bf16, tag=f"a1_{cb}", name=f"a1_{cb}")
                            av = a1.rearrange("p (i j) -> p i j", j=H)
                            # pass 1 (j): A1 = a1_true / 0.75
                            if pool_side:
                                tm1 = ph2.tile([128, H2 * H2], bf16, tag=f"tm1_{cb}", name=f"tm1_{cb}")
                                t1v = tm1.rearrange("p (i j) -> p i j", j=H2)
                                nc.gpsimd.tensor_mul(t1v[:, :, 0:27],
                                                     xv[:, :, 0:27], tv[:, :, 0:27])
                                nc.gpsimd.tensor_add(av[:, :, 2:55:2],
                                                     t1v[:, :, 0:27], xv[:, :, 1:28])
                                nc.gpsimd.tensor_mul(t1v[:, :, 1:28],
                                                     xv[:, :, 1:28], tv[:, :, 0:27])
                                nc.gpsimd.tensor_add(av[:, :, 1:54:2],
                                                     t1v[:, :, 1:28], xv[:, :, 0:27])
                            else:
                                nc.vector.scalar_tensor_tensor(
                                    out=av[:, :, 2:55:2], in0=xv[:, :, 0:27],
                                    scalar=1.0 / 3.0, in1=xv[:, :, 1:28],
                                    op0=OP.mult, op1=OP.add)
                                nc.vector.scalar_tensor_tensor(
                                    out=av[:, :, 1:54:2], in0=xv[:, :, 1:28],
                                    scalar=1.0 / 3.0, in1=xv[:, :, 0:27],
                                    op0=OP.mult, op1=OP.add)
                            nc.scalar.activation(out=av[:, :, 0:1],
                                                 in_=xv[:, :, 0:1],
                                                 func=AF.Copy, scale=4.0 / 3.0)
                            nc.scalar.activation(out=av[:, :, 55:56],
                                                 in_=xv[:, :, 27:28],
                                                 func=AF.Copy, scale=4.0 / 3.0)
                            # pass 2 (i): U = u_true / 0.5625
                            uv = u3[cb].rearrange("p (i j) -> p i j", j=H)
                            if pool_side:
                                tm2 = ph2.tile([128, H2 * H], bf16, tag=f"tm2_{cb}", name=f"tm2_{cb}")
                                t2v = tm2.rearrange("p (i j) -> p i j", j=H)
                                nc.gpsimd.tensor_mul(t2v[:, 0:27, :],
                                                     av[:, 0:27, :], tv[:, 0:27, :])
                                nc.gpsimd.tensor_add(uv[:, 2:55:2, :],
                                                     t2v[:, 0:27, :], av[:, 1:28, :])
                                nc.gpsimd.tensor_mul(t2v[:, 1:28, :],
                                                     av[:, 1:28, :], tv[:, 0:27, :])
                                nc.gpsimd.tensor_add(uv[:, 1:54:2, :],
                                                     t2v[:, 1:28, :], av[:, 0:27, :])
                            else:
                                nc.vector.scalar_tensor_tensor(
                                    out=uv[:, 2:55:2, :], in0=av[:, 0:27, :],
                                    scalar=1.0 / 3.0, in1=av[:, 1:28, :],
                                    op0=OP.mult, op1=OP.add)
                                nc.vector.scalar_tensor_tensor(
                                    out=uv[:, 1:54:2, :], in0=av[:, 1:28, :],
                                    scalar=1.0 / 3.0, in1=av[:, 0:27, :],
                                    op0=OP.mult, op1=OP.add)
                            nc.scalar.activation(out=uv[:, 0:1, :],
                                                 in_=av[:, 0:1, :],
                                                 func=AF.Copy, scale=4.0 / 3.0)
                            nc.scalar.activation(out=uv[:, 55:56, :],
                                                 in_=av[:, 27:28, :],
                                                 func=AF.Copy, scale=4.0 / 3.0)

                    PHASE_MARKS.append(('ph5', nc._state.next_id()))
                    # ---- Phase 5-own (staged into loop A) ----
                    import contextlib
                    _lab_es = contextlib.ExitStack()
                    if True:
                        ph5s = _lab_es.enter_context(tc.tile_pool(name="ph5s", bufs=1))
                        ph5 = _lab_es.enter_context(tc.tile_pool(name="ph5", bufs=1))
                        ph5w = _lab_es.enter_context(tc.tile_pool(name="ph5w", bufs=1))
                        ph5x = _lab_es.enter_context(tc.tile_pool(name="ph5x", bufs=1))
                        _ph5ps_es = contextlib.ExitStack()
                        ph5p = _ph5ps_es.enter_context(
                            tc.tile_pool(name="ph5p", bufs=1, space="PSUM"))
                        ph5q = _ph5ps_es.enter_context(
                            tc.tile_pool(name="ph5q", bufs=1, space="PSUM"))
                        tqh = load_w3(ph5s, w_qhT, "qhT")
                        bqh3 = load_b3(ph5s, v_bqh, "bqh")
                        tql = load_w3(ph5s, w_qlT, "qlT")
                        bql3 = load_b3(ph5s, v_bql, "bql")
                        p2upo = [ph5s.tile([128, QPC], bf16, tag=f"p2upo{c}", name=f"p2upo{c}")
                                 for c in range(CT)]
                        p1no = [ph5s.tile([128, QPC], bf16, tag=f"p1no{c}", name=f"p1no{c}")
                                for c in range(CT)]
                        p1o5 = [ph5x.tile([128, QPC], bf16, tag=f"p1o5{c}", name=f"p1o5{c}")
                                for c in range(CT)]
                        for cb in range(CT):
                            nc.gpsimd.dma_start(
                                out=p1o5[cb],
                                in_=p1T_own[cb * 128:(cb + 1) * 128, :].bitcast(f32))

                        def ln_full5(pool, ppool, w, y3, dst_ap):
                            ps_m = ppool.tile([128, w], f32, tag="ln_psm", name="ln_psm")
                            for cb in range(CT):
                                nc.tensor.matmul(ps_m, ones_b, y3[cb],
                                                 start=(cb == 0), stop=(cb == CT - 1))
                            ps_s = ppool.tile([128, w], f32, tag="ln_pss", name="ln_pss")
                            for cb in range(CT):
                                sq = pool.tile([128, w], bf16, tag="ln_sq", name="ln_sq")
                                if cb == 0:
                                    nc.scalar.activation(out=sq, in_=y3[cb], func=AF.Square)
                                else:
                                    nc.gpsimd.tensor_mul(sq, y3[cb], y3[cb])
                                nc.tensor.matmul(ps_s, ones_b, sq,
                                                 start=(cb == 0), stop=(cb == CT - 1))
                            m_bc = pool.tile([128, w], bf16, tag="ln_mbc", name="ln_mbc")
                            nc.scalar.activation(out=m_bc, in_=ps_m, func=AF.Copy,
                                                 scale=1.0 / C)
                            m2 = pool.tile([128, w], bf16, tag="ln_m2", name="ln_m2")
                            nc.gpsimd.tensor_mul(m2, m_bc, m_bc)
                            v_bc = pool.tile([128, w], f32, tag="ln_vbc", name="ln_vbc")
                            nc.vector.scalar_tensor_tensor(
                                out=v_bc, in0=ps_s, scalar=1.0 / C, in1=m2,
                                op0=OP.mult, op1=OP.subtract)
                            sd = pool.tile([128, w], f32, tag="ln_sd", name="ln_sd")
                            nc.scalar.activation(out=sd, in_=v_bc, func=AF.Sqrt,
                                                 bias=eps_t, scale=1.0)
                            r_bc = pool.tile([128, w], f32, tag="ln_rbc", name="ln_rbc")
                            nc.vector.reciprocal_approx_fast(out=r_bc, in_=sd)
                            r16 = pool.tile([128, w], bf16, tag="ln_r16", name="ln_r16")
                            nc.scalar.activation(out=r16, in_=r_bc, func=AF.Copy)
                            for cb in range(CT):
                                xc = pool.tile([128, w], bf16, tag="ln_xc", name="ln_xc")
                                nc.gpsimd.tensor_sub(xc, y3[cb], m_bc)
                                nc.vector.tensor_mul(dst_ap(cb), xc, r16)

                        def ln_zm5(pool, ppool, w, y3, dst_ap):
                            ps_s = ppool.tile([128, w], f32, tag="ln_pss", name="lz_pss")
                            for cb in range(CT):
                                sq = pool.tile([128, w], bf16, tag="ln_sq", name="lz_sq")
                                if cb == 0:
                                    nc.scalar.activation(out=sq, in_=y3[cb], func=AF.Square)
                                else:
                                    nc.gpsimd.tensor_mul(sq, y3[cb], y3[cb])
                                nc.tensor.matmul(ps_s, ones_b, sq,
                                                 start=(cb == 0), stop=(cb == CT - 1))
                            sd = pool.tile([128, w], f32, tag="ln_sd", name="lz_sd")
                            nc.scalar.activation(out=sd, in_=ps_s, func=AF.Sqrt,
                                                 bias=eps_t, scale=1.0 / C)
                            r_bc = pool.tile([128, w], f32, tag="ln_rbc", name="lz_rbc")
                            nc.vector.reciprocal_approx_fast(out=r_bc, in_=sd)
                            r16 = pool.tile([128, w], bf16, tag="ln_r16", name="lz_r16")
                            nc.scalar.activation(out=r16, in_=r_bc, func=AF.Copy)
                            for cb in range(CT):
                                nc.vector.tensor_mul(dst_ap(cb), y3[cb], r16)

                        ph5_stages = []

                        def st_up(qc2):
                            qsl = slice(qc2 * QC, (qc2 + 1) * QC)
                            wts = []
                            for kt in range(TOK2):
                                wt = ph5w.tile([TT2, QC], bf16,
                                               tag=f"wupo{kt}", name=f"wupo{kt}")
                                nc.gpsimd.dma_start(
                                    out=wt,
                                    in_=WupT_own[kt * TT2:(kt + 1) * TT2, qsl])
                                wts.append(wt)
                            uo3 = []
                            for cb in range(CT):
                                ps = ph5q.tile([128, QC], f32, tag="ps_upo", name="ps_upo")
                                for kt in range(TOK2):
                                    nc.tensor.matmul(
                                        ps,
                                        xn7[kt][:, cb * 128:(cb + 1) * 128],
                                        wts[kt],
                                        start=(kt == 0), stop=(kt == TOK2 - 1))
                                uo = ph5.tile([128, QC], bf16, tag=f"uo{cb}", name=f"uo{cb}")
                                nc.vector.tensor_copy(uo, ps)
                                uo3.append(uo)
                            return uo3, qsl

                        def st_zm(qc2):
                            uo3, qsl = st_up(qc2)
                            ln_zm5(ph5, ph5p, QC, uo3,
                                   lambda cb, _sl=qsl: p2upo[cb][:, _sl])
                            ln_full5(ph5, ph5p, QC,
                                     [p1o5[cb][:, qsl] for cb in range(CT)],
                                     lambda cb, _sl=qsl: p1no[cb][:, _sl])

                        def st_q(a, qc2):
                            dst, qsrc, tw, tb = ((qh, p1no, tqh, bqh3),
                                                 (ql, p2upo, tql, bql3))[a]
                            if True:
                                qsl = slice(qc2 * QC, (qc2 + 1) * QC)
                                for cb in range(CT):
                                    ps = ph5q.tile([128, QC], f32, tag="ps_upo", name="ps_q")
                                    for kt in range(CT):
                                        nc.tensor.matmul(
                                            ps,
                                            tw[kt][:, cb * 128:(cb + 1) * 128],
                                            qsrc[kt][:, qsl],
                                            start=(kt == 0),
                                            stop=(kt == CT - 1))
                                    nc.scalar.activation(
                                        out=dst[cb][:, qsl], in_=ps,
                                        func=AF.Identity, bias=tb[cb],
                                        scale=1.0)

                        ph5_stages = [
                            lambda: st_zm(0), lambda: st_zm(1),
                            lambda: st_q(0, 0), lambda: st_q(0, 1),
                            lambda: st_q(1, 0), lambda: st_q(1, 1)]

                    PHASE_MARKS.append(('stats', nc._state.next_id()))
                    # ---- Loop A (Sqrt era): LN stats + fp8 value tiles +
                    #      high-branch K/V, chunk-streamed ----
                    ph6w = _lab_es.enter_context(tc.tile_pool(name="ph6w", bufs=1))
                    ph6v = _lab_es.enter_context(tc.tile_pool(name="ph6v", bufs=7))
                    ph6g = _lab_es.enter_context(tc.tile_pool(name="ph6g", bufs=2))
                    tkh = load_pk(ph6w, w_khP, "khP")
                    tkl = load_pk(ph6w, w_klP, "klP")
                    tvh = load_pk(ph6w, w_vhP, "vhP")
                    tvl = load_pk(ph6w, w_vlP, "vlP")
                    tl1L = load_pk(ph6w, w_pl1LP, "pl1LP")
                    tl1R = load_pk(ph6w, w_pl1RP, "pl1RP")
                    tl1M = load_pk(ph6w, w_pl1MX, "pl1MX")
                    bl1 = load_b3(ph6w, v_pl1b, "pl1b")
                    PUs, PNs, MXs = {}, {}, {}
                    with tc.tile_pool(name="sts", bufs=2) as sts, \
                         tc.tile_pool(name="sty", bufs=2) as sty, \
                         tc.tile_pool(name="ph7p", bufs=2, space="PSUM") as ph7p, \
                         tc.tile_pool(name="ph7v", bufs=1, space="PSUM") as ph7v, \
                         tc.tile_pool(name="stp", bufs=2, space="PSUM") as stp:
                        for (c0, cw) in CHUNKS:
                            csl = slice(c0, c0 + cw)
                            # u-LN (zero-mean): r_u
                            ps_s = stp.tile([128, 512], f32, tag="st_ps", name="st_psu")[:, :cw]
                            for cb in range(CT):
                                sq = sts.tile([128, 512], bf16, tag=f"st_sq{cb}",
                                              name=f"st_squ{cb}")[:, :cw]
                                if cb == 0:
                                    nc.vector.tensor_mul(sq, u3[cb][:, csl],
                                                         u3[cb][:, csl])
                                elif cb == 1:
                                    nc.vector.tensor_mul(sq, u3[cb][:, csl],
                                                         u3[cb][:, csl])
                                else:
                                    nc.gpsimd.tensor_mul(sq, u3[cb][:, csl],
                                                         u3[cb][:, csl])
                                nc.tensor.matmul(ps_s, ones_b, sq,
                                                 start=(cb == 0), stop=(cb == CT - 1))
                            sd = sts.tile([128, 512], f32, tag="st_sd", name="st_sdu")[:, :cw]
                            nc.scalar.activation(out=sd, in_=ps_s, func=AF.Sqrt,
                                                 bias=eps_t, scale=1.0 / C)
                            rf = sts.tile([128, 512], f32, tag="st_rf", name="st_rfu")[:, :cw]
                            nc.vector.reciprocal_approx_fast(out=rf, in_=sd)
                            nc.gpsimd.tensor_scalar_mul(out=r_u[:, csl], in0=rf,
                                                        scalar1=1.0)
                            # p1-LN stats: m_p1, r_p1
                            y3 = []
                            for cb in range(CT):
                                yb = sty.tile([128, 512], bf16,
                                              tag=f"st_y{cb}", name=f"st_y{cb}")[:, :cw]
                                nc.gpsimd.dma_start(
                                    out=yb,
                                    in_=p1T[cb * 128:(cb + 1) * 128, csl].bitcast(f32))
                                y3.append(yb)
                            ps_m = stp.tile([128, 512], f32, tag="st_ps", name="st_psm")[:, :cw]
                            for cb in range(CT):
                                nc.tensor.matmul(ps_m, ones_b, y3[cb],
                                                 start=(cb == 0), stop=(cb == CT - 1))
                            ps_s2 = stp.tile([128, 512], f32, tag="st_ps", name="st_psp")[:, :cw]
                            for cb in range(CT):
                                sq = sts.tile([128, 512], bf16, tag=f"st_sq{cb}",
                                              name=f"st_sqp{cb}")[:, :cw]
                                if cb == 0:
                                    nc.gpsimd.tensor_mul(sq, y3[cb], y3[cb])
                                elif cb == 1:
                                    nc.vector.tensor_mul(sq, y3[cb], y3[cb])
                                else:
                                    nc.gpsimd.tensor_mul(sq, y3[cb], y3[cb])
                                nc.tensor.matmul(ps_s2, ones_b, sq,
                                                 start=(cb == 0), stop=(cb == CT - 1))
                            nc.vector.tensor_scalar_mul(out=m_p1[:, csl],
                                                         in0=ps_m, scalar1=1.0 / C)
                            m2 = sts.tile([128, 512], bf16, tag="st_m2", name="st_m2")[:, :cw]
                            nc.gpsimd.tensor_mul(m2, m_p1[:, csl], m_p1[:, csl])
                            v_bc = sts.tile([128, 512], f32, tag="st_v", name="st_v")[:, :cw]
                            nc.vector.scalar_tensor_tensor(
                                out=v_bc, in0=ps_s2, scalar=1.0 / C, in1=m2,
                                op0=OP.mult, op1=OP.subtract)
                            sd2 = sts.tile([128, 512], f32, tag="st_sd", name="st_sd2")[:, :cw]
                            nc.scalar.activation(out=sd2, in_=v_bc, func=AF.Sqrt,
                                                 bias=eps_t, scale=1.0)
                            rf2 = sts.tile([128, 512], f32, tag="st_rf", name="st_rfp")[:, :cw]
                            nc.vector.reciprocal_approx_fast(out=rf2, in_=sd2)
                            nc.gpsimd.tensor_scalar_mul(out=r_p1[:, csl], in0=rf2,
                                                        scalar1=1.0)
                            # fp8 paired value tiles (persist through loop B)
                            PU = ph6v.tile([128, 2, 512], fp8, tag="PU", name="PU")[:, :, :cw]
                            PN = ph6v.tile([128, 2, 512], fp8, tag="PN", name="PN")[:, :, :cw]
                            MX = ph6v.tile([128, 2, 512], fp8, tag="MX", name="MX")[:, :, :cw]
                            PUs[c0], PNs[c0], MXs[c0] = PU, PN, MX
                            nc.vector.tensor_mul(PU[:, 0, :], u3[0][:, csl], r_u[:, csl])
                            nc.vector.tensor_mul(PU[:, 1, :], u3[1][:, csl], r_u[:, csl])
                            nc.gpsimd.tensor_mul(MX[:, 0, :], u3[2][:, csl], r_u[:, csl])
                            for cb in range(CT):
                                xc = sts.tile([128, 512], bf16, tag=f"st_sq{cb}",
                                              name=f"xc{cb}")[:, :cw]
                                nc.gpsimd.tensor_sub(xc, y3[cb], m_p1[:, csl])
                                dstv = PN[:, cb, :] if cb < 2 else MX[:, 1, :]
                                if cb == 1:
                                    nc.gpsimd.tensor_mul(dstv, xc, r_p1[:, csl])
                                else:
                                    nc.vector.tensor_mul(dstv, xc, r_p1[:, csl])
                            # high-branch K + V (Copy evacs live in any table)
                            for cb in range(CT):
                                osl = slice(cb * 128, (cb + 1) * 128)
                                ps = ph7p.tile([128, 512], f32, tag="ps_k", name="ps_k")[:, :cw]
                                nc.tensor.matmul(ps, tkh[:, 0:2, osl],
                                                 PU[:, 0:2, :],
                                                 start=True, stop=False,
                                                 perf_mode=DR)
                                nc.tensor.matmul(ps, tkh[:, 2, osl],
                                                 MX[:, 0, :],
                                                 start=False, stop=True)
                                nc.scalar.activation(
                                    out=kh[cb][:, csl], in_=ps,
                                    func=AF.Copy, scale=r_kh)
                            t = c0
                            pi = 0
                            while t < c0 + cw:
                                vi, vj = t // 128, t % 128
                                jn = min(128 - vj, c0 + cw - t)
                                tsl = slice(t - c0, t - c0 + jn)
                                ps = ph7v.tile([128, C], f32, tag="ps_v", name="ps_v")
                                if jn == 128:
                                    nc.tensor.matmul(
                                        ps[vj:vj + jn], PU[:, 0:2, tsl],
                                        tvh[:, 0:2, :], start=True, stop=False,
                                        perf_mode=DR)
                                else:
                                    nc.tensor.matmul(
                                        ps[vj:vj + jn], PU[:, 0, tsl],
                                        tvh[:, 0, :], start=True, stop=False)
                                    nc.tensor.matmul(
                                        ps[vj:vj + jn], PU[:, 1, tsl],
                                        tvh[:, 1, :], start=False, stop=False)
                                nc.tensor.matmul(
                                    ps[vj:vj + jn], MX[:, 0, tsl],
                                    tvh[:, 2, :], start=False, stop=True)
                                vt, slot = vslot(0, vi)
                                if slot is None:
                                    dstv = vt[vj:vj + jn, :]
                                else:
                                    dstv = vt[vj:vj + jn, slot, :]
                                if pi % 2 == 0:
                                    nc.vector.tensor_scalar_mul(
                                        out=dstv, in0=ps[vj:vj + jn], scalar1=r_vh)
                                else:
                                    nc.scalar.activation(
                                        out=dstv, in_=ps[vj:vj + jn],
                                        func=AF.Copy, scale=r_vh)
                                t += jn
                                pi += 1
                            if ph5_stages:
                                ph5_stages.pop(0)()

                    while ph5_stages:
                        ph5_stages.pop(0)()
                    _ph5ps_es.close()
                    PHASE_MARKS.append(('ph4', nc._state.next_id()))
                    ph7pb = _lab_es.enter_context(
                        tc.tile_pool(name="ph7pb", bufs=2, space="PSUM"))
                    ph7vb = _lab_es.enter_context(
                        tc.tile_pool(name="ph7vb", bufs=2, space="PSUM"))
                    # ---- Loop B (Gelu era): pl1 -> gelu -> low-branch K/V ----
                    for (c0, cw) in CHUNKS:
                        csl = slice(c0, c0 + cw)
                        PU, PN, MX = PUs[c0], PNs[c0], MXs[c0]
                        GL = ph6g.tile([128, 3, 512], fp8, tag="GL", name="GL")[:, :, :cw]
                        for cb in range(CT):
                            osl = slice(cb * 128, (cb + 1) * 128)
                            ps = ph7pb.tile([128, 512], f32, tag="ps_pp1", name="ps_pp1")[:, :cw]
                            nc.tensor.matmul(ps, tl1L[:, :, osl], PN,
                                             start=True, stop=False, perf_mode=DR)
                            nc.tensor.matmul(ps, tl1R[:, :, osl], PU,
                                             start=False, stop=False, perf_mode=DR)
                            nc.tensor.matmul(ps, tl1M[:, :, osl], MX,
                                             start=False, stop=True, perf_mode=DR)
                            nc.scalar.activation(out=GL[:, cb, :], in_=ps,
                                                 func=AF.Gelu,
                                                 bias=bl1[cb], scale=r_pl1)
                        for cb in range(CT):
                            osl = slice(cb * 128, (cb + 1) * 128)
                            ps = ph7pb.tile([128, 512], f32, tag="ps_k", name="ps_k")[:, :cw]
                            nc.tensor.matmul(ps, tkl[:, 0:2, osl],
                                             GL[:, 0:2, :],
                                             start=True, stop=False, perf_mode=DR)
                            nc.tensor.matmul(ps, tkl[:, 2, osl],
                                             GL[:, 2, :],
                                             start=False, stop=True)
                            nc.vector.tensor_scalar_mul(
                                out=kl[cb][:, csl], in0=ps, scalar1=r_kl)
                        t = c0
                        pi = 0
                        while t < c0 + cw:
                            vi, vj = t // 128, t % 128
                            jn = min(128 - vj, c0 + cw - t)
                            tsl = slice(t - c0, t - c0 + jn)
                            ps = ph7vb.tile([128, C], f32, tag="ps_v", name="ps_v")
                            if jn == 128:
                                nc.tensor.matmul(
                                    ps[vj:vj + jn], GL[:, 0:2, tsl],
                                    tvl[:, 0:2, :], start=True, stop=False,
                                    perf_mode=DR)
                            else:
                                nc.tensor.matmul(
                                    ps[vj:vj + jn], GL[:, 0, tsl],
                                    tvl[:, 0, :], start=True, stop=False)
                                nc.tensor.matmul(
                                    ps[vj:vj + jn], GL[:, 1, tsl],
                                    tvl[:, 1, :], start=False, stop=False)
                            nc.tensor.matmul(
                                ps[vj:vj + jn], GL[:, 2, tsl],
                                tvl[:, 2, :], start=False, stop=True)
                            vt, slot = vslot(1, vi)
                            if slot is None:
                                dstv = vt[vj:vj + jn, :]
                            else:
                                dstv = vt[vj:vj + jn, slot, :]
                            if pi % 2 == 0:
                                nc.scalar.activation(
                                    out=dstv, in_=ps[vj:vj + jn],
                                    func=AF.Copy, scale=r_vl)
                            else:
                                nc.vector.tensor_scalar_mul(
                                    out=dstv, in0=ps[vj:vj + jn], scalar1=r_vl)
                            t += jn
                            pi += 1
                    _lab_es.close()

            PHASE_MARKS.append(('ph8', nc._state.next_id()))
            # ---- Phase 8: attention (Exp era) ----
            with tc.tile_pool(name="outs", bufs=1) as outsp:
                oh = [outsp.tile([128, QPC], f32r, tag=f"oh{c}", name=f"oh{c}")
                      for c in range(CT)]
                ol = [outsp.tile([128, QPC], f32r, tag=f"ol{c}", name=f"ol{c}")
                      for c in range(CT)]
                with tc.tile_pool(name="atw", bufs=1) as atw:
                    tfoh = load_w3g(atw, w_fohT, "fohT")
                    bfoh = load_b3(atw, v_fohb, "fohb")
                    tfol = load_w3g(atw, w_folT, "folT")
                    bfol = load_b3(atw, v_folb, "folb")

                    with tc.tile_pool(name="ate", bufs=1) as ate, \
                         tc.tile_pool(name="at", bufs=4) as at, \
                         tc.tile_pool(name="atee", bufs=4) as atee, \
                         tc.tile_pool(name="ata", bufs=2) as ata, \
                         tc.tile_pool(name="ats", bufs=3) as ats, \
                         tc.tile_pool(name="ato", bufs=1) as ato, \
                         tc.tile_pool(name="atps", bufs=2, space="PSUM") as atps, \
                         tc.tile_pool(name="atpo", bufs=2, space="PSUM") as atpo, \
                         tc.tile_pool(name="atpd", bufs=2, space="PSUM") as atpd:
                        onorm_all = {}
                        pending = []   # deferred AV/denom matmul thunks
                        prev_fin = None

                        def emit_pend(k):
                            for _ in range(k):
                                if not pending:
                                    break
                                pending.pop(0)()

                        ebts = {}
                        for qc in range(NQC):
                            qsl = slice(qc * QC, (qc + 1) * QC)
                            for h in range(HEADS):
                                ebt = ate.tile([128, 25, QC], fp8,
                                               tag=f"ebt{(3 * qc + h) % 2}",
                                               name=f"ebt{h}{qc}")
                                nc.sync.dma_start(out=ebt, in_=expB8[h, qc])
                                ps_o = [atpo.tile([128, QC], f32, tag="ps_o", name="ps_o")
                                        for _ in range(2)]
                                ps_d = [atpd.tile([128, QC], f32, tag="ps_d", name="ps_d")
                                        for _ in range(2)]
                                aa_pair = {}
                                a24 = {}
                                for i, (j0, jn) in enumerate(JTS):
                                    ps2 = atps.tile([128, 1024], f32, tag="ps2", name="ps2")
                                    nc.tensor.matmul(
                                        ps2[:jn, 0:QC], kh[h][:, j0:j0 + jn],
                                        qh[h][:, qsl], start=True, stop=True)
                                    nc.tensor.matmul(
                                        ps2[:jn, 512:512 + QC], kl[h][:, j0:j0 + jn],
                                        ql[h][:, qsl], start=True, stop=True)
                                    # drain deferred PE work under the exp
                                    emit_pend(1)
                                    ee = atee.tile([128, 2 * QC], bf16, tag="ee", name="ee")
                                    nc.scalar.activation(
                                        out=ee[:jn].rearrange("p (b x) -> p b x", b=2),
                                        in_=ps2.rearrange(
                                            "p (b x) -> p b x", b=2)[:jn, :, 0:QC],
                                        func=AF.Exp)
                                    ebs = ebt[:jn, i, :]
                                    if i < 24:
                                        p, slot = i // 2, i % 2
                                        if slot == 0:
                                            aah = ata.tile([128, 2, QC], fp8,
                                                           tag=f"aah{p}", name=f"aah{p}")
                                            aal = ata.tile([128, 2, QC], fp8,
                                                           tag=f"aal{p}", name=f"aal{p}")
                                            aa_pair[p] = (aah, aal)
                                        else:
                                            aah, aal = aa_pair[p]
                                        nc.vector.tensor_mul(
                                            aah[:jn, slot, :], ee[:jn, 0:QC], ebs)
                                        nc.gpsimd.tensor_mul(
                                            aal[:jn, slot, :], ee[:jn, QC:2 * QC], ebs)
                                    else:
                                        aah24 = ats.tile([64, QC], fp8,
                                                         tag="aah24", name="aah24")
                                        aal24 = ats.tile([64, QC], fp8,
                                                         tag="aal24", name="aal24")
                                        nc.vector.tensor_mul(
                                            aah24, ee[:jn, 0:QC], ebs)
                                        nc.gpsimd.tensor_mul(
                                            aal24, ee[:jn, QC:2 * QC], ebs)
                                        a24 = {0: aah24, 1: aal24}
                                # build this iteration's deferred sweep
                                while pending:        # safety: drain leftovers
                                    pending.pop(0)()

                                def mk_step(p, _po=ps_o, _pd=ps_d, _ap=aa_pair,
                                            _h=h):
                                    def go():
                                        for a, vps in ((0, vhp), (1, vlp)):
                                            aa = _ap[p][a]
                                            nc.tensor.matmul(
                                                _po[a],
                                                vps[p][:, :, _h * 128:(_h + 1) * 128],
                                                aa, start=(p == 0), stop=False,
                                                perf_mode=DR)
                                            nc.tensor.matmul(
                                                _pd[a], ones8p, aa,
                                                start=(p == 0), stop=False,
                                                perf_mode=DR)
                                    return go

                                def mk_last(_po=ps_o, _pd=ps_d, _a24=a24, _h=h):
                                    def go():
                                        for a, v24 in ((0, vh24), (1, vl24)):
                                            aa = _a24[a]
                                            nc.tensor.matmul(
                                                _po[a],
                                                v24[:, _h * 128:(_h + 1) * 128],
                                                aa, start=False, stop=True)
                                            nc.tensor.matmul(
                                                _pd[a], ones8p[:64, 0, :], aa,
                                                start=False, stop=True)
                                    return go

                                def mk_fin(_po=ps_o, _pd=ps_d, _qc=qc, _h=h):
                                    def go():
                                        for a in range(2):
                                            rden = at.tile([128, QC], f32,
                                                           tag="rden", name="rden")
                                            nc.vector.reciprocal_approx_fast(
                                                out=rden, in_=_pd[a])
                                            on = ato.tile(
                                                [128, QC], bf16,
                                                tag=f"on{a}{_h}{_qc}",
                                                name=f"on{a}{_h}{_qc}")
                                            nc.vector.tensor_mul(on, _po[a], rden)
                                            onorm_all[(_qc, a, _h)] = on
                                    return go

                                def mk_oproj(a, cb, _qc=qc, _qsl=qsl):
                                    def go():
                                        dst, tw, tb = ((oh, tfoh, bfoh),
                                                       (ol, tfol, bfol))[a]
                                        ps = atpd.tile([128, QC], f32,
                                                       tag="ps_d", name="ps_dp")
                                        for h2 in range(HEADS):
                                            nc.tensor.matmul(
                                                ps,
                                                tw[h2][:, cb * 128:(cb + 1) * 128],
                                                onorm_all[(_qc, a, h2)],
                                                start=(h2 == 0),
                                                stop=(h2 == HEADS - 1))
                                        nc.vector.tensor_scalar_add(
                                            out=dst[cb][:, _qsl], in0=ps,
                                            scalar1=tb[cb])
                                    return go

                                pending = [mk_step(p) for p in range(NPAIR)]
                                pending.append(mk_last())
                                pending.append(mk_fin())
                                prev_fin = None
                                if h == HEADS - 1:
                                    for a in range(2):
                                        for cb in range(CT):
                                            pending.append(mk_oproj(a, cb))
                        # drain the final qc's deferred work
                        while pending:
                            pending.pop(0)()
                PHASE_MARKS.append(('ph9', nc._state.next_id()))
                # ---- Phase 9: gate, mix, ff ----
                with tc.tile_pool(name="ph9w", bufs=1) as ph9w, \
                     tc.tile_pool(name="ph9", bufs=2) as ph9, \
                     tc.tile_pool(name="ph9p", bufs=2, space="PSUM") as ph9p:
                    tg1L = load_w3(ph9w, w_g1LT, "g1LT")
                    tg1R = load_w3(ph9w, w_g1RT, "g1RT")
                    bg1 = load_b3(ph9w, v_g1b, "g1b")
                    tg2 = load_w3(ph9w, w_g2T, "g2T")
                    g2b_t = ph9w.tile([1, 1], f32, tag="g2b_t", name="g2b_t")
                    nc.sync.dma_start(
                        out=g2b_t, in_=v_g2b.rearrange("(a b) -> a b", a=1))
                    ones_f = ph9w.tile([1, 128], f32r, tag="ones_f1", name="ones_f1")
                    nc.sync.dma_start(out=ones_f, in_=ones_row_d)
                    tffL = load_w3(ph9w, w_ffLT, "ffLT")
                    tffP = load_w3(ph9w, w_ffPT, "ffPT")
                    bff = load_b3(ph9w, v_ffb, "ffb")
                    p1o = [ph9w.tile([128, QPC], f32r, tag=f"p1o{c}", name=f"p1o{c}")
                           for c in range(CT)]
                    for cb in range(CT):
                        nc.sync.dma_start(
                            out=p1o[cb],
                            in_=p1T_own[cb * 128:(cb + 1) * 128, :])
                    qsls = [slice(qc * QC, (qc + 1) * QC) for qc in range(NQC)]
                    for qc in range(NQC):
                        gel = {}
                        for cb in range(CT):
                            ps = ph9p.tile([128, QC], f32, tag="ps_g1", name="ps_g1")
                            for kt in range(CT):
                                nc.tensor.matmul(
                                    ps,
                                    r32(tg1L[kt][:, cb * 128:(cb + 1) * 128]),
                                    r32(oh[kt][:, qsls[qc]]),
                                    start=(kt == 0), stop=False)
                            for kt in range(CT):
                                nc.tensor.matmul(
                                    ps,
                                    r32(tg1R[kt][:, cb * 128:(cb + 1) * 128]),
                                    r32(ol[kt][:, qsls[qc]]), start=False,
                                    stop=(kt == CT - 1))
                            gt = ph9.tile([128, QC], f32r, tag=f"ggel{qc}{cb}",
                                          name=f"ggel{qc}{cb}")
                            nc.scalar.activation(out=gt, in_=ps, func=AF.Gelu,
                                                 bias=bg1[cb], scale=1.0)
                            gel[cb] = gt
                        ps_z = ph9p.tile([1, QC], f32, tag="ps_z", name="ps_z")
                        for kt in range(CT):
                            nc.tensor.matmul(ps_z, r32(tg2[kt]), r32(gel[kt]),
                                             start=(kt == 0),
                                             stop=(kt == CT - 1))
                        gate = ph9.tile([1, QC], f32r, tag="gate", name="gate")
                        nc.scalar.activation(out=gate, in_=ps_z,
                                             func=AF.Sigmoid,
                                             bias=g2b_t, scale=1.0)
                        ps_gb = ph9p.tile([128, QC], f32, tag="ps_gb", name="ps_gb")
                        nc.tensor.matmul(ps_gb, r32(ones_f), r32(gate),
                                         start=True, stop=True)
                        mix = {}
                        for cb in range(CT):
                            dd = ph9.tile([128, QC], f32, tag="dd", name="dd")
                            nc.vector.tensor_sub(dd, oh[cb][:, qsls[qc]],
                                                 ol[cb][:, qsls[qc]])
                            d2 = ph9.tile([128, QC], f32, tag="d2", name="d2")
                            nc.vector.tensor_mul(d2, dd, ps_gb)
                            mx = ph9.tile([128, QC], f32r, tag=f"mix{qc}{cb}",
                                          name=f"mix{qc}{cb}")
                            nc.vector.tensor_add(mx, d2, ol[cb][:, qsls[qc]])
                            mix[cb] = mx
                        for cb in range(CT):
                            ps = ph9p.tile([128, QC], f32, tag="ps_ff", name="ps_ff")
                            for kt in range(CT):
                                nc.tensor.matmul(
                                    ps,
                                    r32(tffL[kt][:, cb * 128:(cb + 1) * 128]),
                                    r32(mix[kt]), start=(kt == 0), stop=False)
                            for kt in range(CT):
                                nc.tensor.matmul(
                                    ps,
                                    r32(tffP[kt][:, cb * 128:(cb + 1) * 128]),
                                    r32(p1o[kt][:, qsls[qc]]), start=False,
                                    stop=(kt == CT - 1))
                            res = ph9.tile([128, QC], f32, tag="res", name="res")
                            nc.scalar.activation(out=res, in_=ps,
                                                 func=AF.Identity,
                                                 bias=bff[cb], scale=1.0)
                            nc.gpsimd.dma_start(
                                out=outT[cb * 128:(cb + 1) * 128, qsls[qc]],
                                in_=res)

    nc.compile()
    return nc


def _prepare(inputs):
    """Host prep + input sharding. Returns (nc, in_maps)."""
    global _COMPILED, _COMPILED_KEY
    inp = {k: np.asarray(v) for k, v in inputs.items()}
    g = _host_prep(inp)

    key = tuple(sorted(g["scales"].items()))
    if _COMPILED is None or _COMPILED_KEY != key:
        _COMPILED = _build(g["scales"])
        _COMPILED_KEY = key
    nc = _COMPILED

    p1 = inp["p1"].astype(np.float32)
    p2 = inp["p2"].astype(np.float32)
    import ml_dtypes
    bf16 = ml_dtypes.bfloat16
    shared = {
        "eye128": np.eye(128, dtype=bf16),
        "ones_row": np.ones((1, 128), np.float32),
        "w_projP": g["projP"], "v_projb": g["projb"],
        "w_qhT": g["wqhT"], "v_bqh": g["bqh"],
        "w_qlT": g["wqlT"], "v_bql": g["bql"],
        "w_khP": g["khP"], "w_klP": g["klP"],
        "w_vhP": g["vhP"], "w_vlP": g["vlP"],
        "w_pl1LP": g["pl1LP"], "w_pl1RP": g["pl1RP"],
        "w_pl1MX": g["pl1MX"], "v_pl1b": g["pl1b"],
        "w_fohT": g["fohT"], "v_fohb": g["fohb"],
        "w_folT": g["folT"], "v_folb": g["folb"],
        "w_g1LT": g["g1LT"], "w_g1RT": g["g1RT"], "v_g1b": g["g1b"],
        "w_g2T": g["g2T"], "v_g2b": g["g2b"],
        "w_ffLT": g["ffLT"], "w_ffPT": g["ffPT"], "v_ffb": g["ffb"],
    }
    shared = {k: np.ascontiguousarray(v) for k, v in shared.items()}

    in_maps = []
    for core in range(NCORES):
        b, qi = divmod(core, 4)
        q0 = qi * QPC
        m = dict(shared)
        m["p1T"] = np.ascontiguousarray(p1[b].T)
        m["p1T_own"] = np.ascontiguousarray(p1[b, q0:q0 + QPC, :].T)
        m["p2P"] = np.ascontiguousarray(
            _q8(p2[b].T.reshape(6, 128, L2).transpose(1, 0, 2)))
        m["WupT_own"] = np.ascontiguousarray(
            g["Wup"][q0:q0 + QPC, :].T.astype(bf16))
        eb = g["expB8"][:, :, :, q0:q0 + QPC]           # [h, 128, 25, 784]
        eb = eb.reshape(HEADS, 128, 25, NQC, QC)
        m["expB8"] = np.ascontiguousarray(
            eb.transpose(0, 3, 1, 2, 4))                 # [h, qc, 128, 25, 392]
        in_maps.append(m)

    return nc, in_maps


def _run(nc, in_maps):
    from concourse.bass_utils import run_bass_kernel_spmd
    res = run_bass_kernel_spmd(nc, in_maps, core_ids=list(range(NCORES)))
    out = np.zeros((B, L, C), np.float32)
    for core in range(NCORES):
        b, qi = divmod(core, 4)
        q0 = qi * QPC
        out[b, q0:q0 + QPC, :] = res.results[core]["outT"].T
    return out


def kernel(**inputs):
    nc, in_maps = _prepare(inputs)
    return _run(nc, in_maps)
